# revision 31
# baseline (speedup 1.0000x reference)
"""Trainium2 Bass kernel for GroupRopeAttention (MQA + RoPE, causal).

Shapes (hardcoded): x (2, 2048, 1024), Wq (1024, 2048) -> 16 heads x 128,
Wk/Wv (1024, 128) single shared K/V head. Output (2, 2048, 2048).

Sharding: core c handles batch c//4 and query heads 4*(c%4)..4*(c%4)+3.
K/V are recomputed per core (no collectives). Each core returns a raw
(4*2048, 129) f32 slab = unnormalized PV output plus the softmax
denominator column; the host divides and reassembles.

Per-core pipeline (one TileContext, everything bf16 except PSUM):
  - xT (e-major x) via 16 hardware XBAR DMA-transposes (no PE transposes)
  - K^T projection d-major + RoPE (rotate-half = constant PermT matmul on
    PE; cos/sin tables are host-precomputed bf16 inputs)
  - V rows computed directly (xt-chunk stationary x Wv moving) into
    [V | ones] j-major slabs
  - per head: Q^T projection + RoPE, then causal attention with a
    lookahead-2 software pipeline: scores (kt_t stationary, 256-col qt
    movings) -> exp on ACT (scale folded) -> causal mask via one DVE
    multiply with a constant [tril | ones | shifted-tril] bf16 tile ->
    PV (pt stationary, [V|1] moving) accumulating output + denominator
    in PSUM, DMA'd raw to DRAM. Q-proj matmuls of the next head fill the
    PE pipeline-drain slots.
"""

import sys
import types

sys.path.insert(0, "/opt/trn_rl_repo")

import numpy as np
import ml_dtypes

BF16 = ml_dtypes.bfloat16

B, L, E = 2, 2048, 1024
NH, HD = 16, 128
N_CORES = 8
HPC = 4          # heads per core
THETA = 10000.0
SCALE = 1.0 / float(np.sqrt(HD))
EC = E // 128    # 8 e-chunks
NJ = L // 128    # 16 j-blocks
NG = L // 256    # 8 i-groups per head

_CACHE = {}


def _ensure_ntff_hook():
    """Register the NTFF profile hook if the image's antenv lacks it."""
    try:
        from antenv.axon_hooks import get_axon_ntff_profile_hook  # noqa: F401
        return
    except ImportError:
        pass
    import antenv

    mod = types.ModuleType("antenv.axon_hooks")
    mod._hook = None

    def set_axon_ntff_profile_hook(h):
        mod._hook = h

    def get_axon_ntff_profile_hook():
        return mod._hook

    mod.set_axon_ntff_profile_hook = set_axon_ntff_profile_hook
    mod.get_axon_ntff_profile_hook = get_axon_ntff_profile_hook
    sys.modules["antenv.axon_hooks"] = mod
    antenv.axon_hooks = mod
    try:
        from trn_agent_boot.trn_boot import _ntff_profile_via_ctypes

        set_axon_ntff_profile_hook(
            _ntff_profile_via_ctypes("/opt/axon/libaxon_pjrt.so")
        )
    except Exception:
        pass


def _host_tables():
    freqs = 1.0 / THETA ** (np.arange(0, HD, 2, dtype=np.float64) / HD)  # (64,)
    t = np.arange(L, dtype=np.float64)
    f = t[:, None] * freqs[None, :]  # (L, 64)
    f = np.repeat(f, 2, axis=-1)  # (L, 128)
    rct = np.ascontiguousarray(np.cos(f).T.astype(BF16))  # (128, L)
    rst = np.ascontiguousarray(np.sin(f).T.astype(BF16))  # (128, L)
    # rot[d] = -src[d+1] for even d, +src[d-1] for odd d, via rot = PermT.T @ src
    permt = np.zeros((HD, HD), dtype=BF16)
    for k in range(HD // 2):
        permt[2 * k, 2 * k + 1] = BF16(1.0)
        permt[2 * k + 1, 2 * k] = BF16(-1.0)
    # causal mask window for the two diagonal j-blocks of an i-group:
    # cols 0:128 -> t==2g block, i-local 0..127: keep i >= j
    # cols 128:256 -> t==2g block, i-local 128..255: always kept
    # cols 256:512 -> t==2g+1 block, i-local 0..255: keep i-128 >= j
    j = np.arange(128)[:, None]
    m1 = (np.arange(128)[None, :] >= j)
    mask = np.concatenate(
        [m1, np.ones((128, 128), bool), m1], axis=1
    ).astype(BF16)  # (128, 384)
    return rct, rst, permt, mask


def _build_program():
    import concourse.bass as bass
    import concourse.mybir as mybir
    import concourse.tile as tile
    from concourse.vector_clock import ScopedClock

    MAX_DRAIN_WAITS = 1

    def _max_inst_waits(inst):
        return 1

    class PatchedTileContext(tile.TileContext):
        # This walrus build rejects >2 sync waits per instruction. After
        # scheduling, hoist excess waits onto preceding nops on the same
        # engine (engines execute in order, so semantics are identical).
        def schedule_and_allocate(self, validate_deps=False):
            ret = super().schedule_and_allocate(validate_deps=validate_deps)
            for blk in self.nc.m.functions[0].blocks:
                new_insts = []
                for inst in blk.instructions:
                    mw = _max_inst_waits(inst)
                    si = inst.sync_info
                    waits = list(si.on_wait) if si and si.on_wait else []
                    if len(waits) > mw:
                        n_extra = len(waits) - mw
                        for i in range(0, n_extra, mw):
                            nop = mybir.InstNoOp(
                                name=self.nc.get_next_instruction_name(),
                                ins=[],
                                outs=[],
                            )
                            nop.engine = inst.engine
                            nop.sync_info = mybir.SyncInfo(
                                on_wait=waits[i : min(i + mw, n_extra)],
                                on_update=[],
                            )
                            self.nc.register_instruction(nop, overwrite=True)
                            new_insts.append(nop)
                        inst.sync_info = mybir.SyncInfo(
                            on_wait=waits[n_extra:],
                            on_update=list(si.on_update or []),
                        )
                    new_insts.append(inst)
                blk.instructions = new_insts
            return ret

        # The tile-exit drain gets the same treatment but must stay last in
        # its engine stream, so split it during emission instead.
        def _drain_and_barrier(self, tick_clock, wait_clock):
            drain_inst = self.nc.sync.drain()
            wait_clock.add_sem_waits(
                drain_inst.ins, ScopedClock({None: tick_clock.global_clock})
            )
            si = drain_inst.ins.sync_info
            waits = list(si.on_wait) if si and si.on_wait else []
            if len(waits) > MAX_DRAIN_WAITS:
                drain_inst.ins.sync_info = mybir.SyncInfo(
                    on_wait=waits[:MAX_DRAIN_WAITS],
                    on_update=list(si.on_update or []),
                )
                for i in range(MAX_DRAIN_WAITS, len(waits), MAX_DRAIN_WAITS):
                    nop = self.nc.sync.nop()
                    nop.ins.sync_info = mybir.SyncInfo(
                        on_wait=waits[i : i + MAX_DRAIN_WAITS], on_update=[]
                    )
            self.nc.all_engine_barrier()
            assert self.sems is not None
            popped = self.nc._tile_sem_poison_stack.pop()
            assert popped is self._sem_poison
            self.nc.clear_and_free_semaphores(
                list(self.sems.allocated().values())
            )
            self.nc.all_engine_barrier()

    f32 = mybir.dt.float32
    bf16 = mybir.dt.bfloat16
    EXP = mybir.ActivationFunctionType.Exp
    MUL = mybir.AluOpType.mult
    ADD = mybir.AluOpType.add

    nc = bass.Bass("TRN2", num_devices=N_CORES)

    x_ext = nc.declare_dram_parameter("x", [L, E], bf16, isOutput=False)
    wq_ext = nc.declare_dram_parameter("wq", [E, HPC * HD], bf16, isOutput=False)
    wk_ext = nc.declare_dram_parameter("wk", [E, HD], bf16, isOutput=False)
    wv_ext = nc.declare_dram_parameter("wv", [E, HD], bf16, isOutput=False)
    rct_ext = nc.declare_dram_parameter("rct", [HD, L], bf16, isOutput=False)
    rst_ext = nc.declare_dram_parameter("rst", [HD, L], bf16, isOutput=False)
    permt_ext = nc.declare_dram_parameter("permt", [HD, HD], bf16, isOutput=False)
    mask_ext = nc.declare_dram_parameter("mask", [128, 384], bf16, isOutput=False)
    out_ext = nc.declare_dram_parameter("out", [HPC * L, HD + 1], f32, isOutput=True)
    import os
    DEBUG = bool(os.environ.get("KERNEL_DEBUG"))
    if DEBUG:
        dbg_mask_ext = nc.declare_dram_parameter(
            "dbg_mask", [128, 512], bf16, isOutput=True
        )
        dbg_pt_ext = nc.declare_dram_parameter(
            "dbg_pt", [128, 1024], bf16, isOutput=True
        )
        dbg_pt1_ext = nc.declare_dram_parameter(
            "dbg_pt1", [128, 1024], bf16, isOutput=True
        )
        dbg_ob_ext = nc.declare_dram_parameter(
            "dbg_ob", [2, 128, 258], f32, isOutput=True
        )

    with PatchedTileContext(nc) as tc:
        with (
            tc.tile_pool(name="const", bufs=1) as constp,
            tc.tile_pool(name="un", bufs=2) as unp,
            tc.tile_pool(name="rot", bufs=2) as rotp,
            tc.tile_pool(name="qt", bufs=2) as qtp,
            tc.tile_pool(name="pt", bufs=4) as ptp,
            tc.tile_pool(name="ost", bufs=3) as ostp,
            tc.tile_pool(name="pbig", bufs=3, space="PSUM") as pbig,
            tc.tile_pool(name="poutA", bufs=1, space="PSUM") as poutpA,
            tc.tile_pool(name="poutB", bufs=1, space="PSUM") as poutpB,
        ):
            # ---- constants: wk/wv first so xT transposes start early ----
            wk_sb = constp.tile([128, EC, HD], bf16, tag="wk")
            nc.sync.dma_start(
                out=wk_sb[:], in_=wk_ext.rearrange("(c p) d -> p c d", p=128)
            )
            wv_sb = constp.tile([128, EC, HD], bf16, tag="wv")
            nc.sync.dma_start(
                out=wv_sb[:], in_=wv_ext.rearrange("(c p) d -> p c d", p=128)
            )

            xt = constp.tile([128, EC, L], bf16, tag="xt")
            vones = constp.tile([128, NJ, HD + 1], bf16, tag="vones")
            kt = constp.tile([128, L], bf16, tag="kt")

            # ---- xT via hardware DMA transpose (per quarter, per e-chunk),
            # alternating between the two HWDGE queues (SP and ACT) ----
            def emit_transposes(h2):
                for ec in range(EC):
                    nc.sync.dma_start(
                        out=xt[:, ec, 1024 * h2 : 1024 * (h2 + 1)],
                        in_=x_ext[
                            1024 * h2 : 1024 * (h2 + 1),
                            128 * ec : 128 * (ec + 1),
                        ],
                        transpose=True,
                    )

            emit_transposes(0)
            wq_sb = constp.tile([128, EC, HPC * HD], bf16, tag="wq")
            nc.sync.dma_start(
                out=wq_sb[:], in_=wq_ext.rearrange("(c p) d -> p c d", p=128)
            )
            emit_transposes(1)
            rct_sb = constp.tile([128, L], bf16, tag="rct")
            nc.sync.dma_start(out=rct_sb[:], in_=rct_ext[:])
            rst_sb = constp.tile([128, L], bf16, tag="rst")
            nc.sync.dma_start(out=rst_sb[:], in_=rst_ext[:])
            permt_sb = constp.tile([128, 128], bf16, tag="permt")
            nc.sync.dma_start(out=permt_sb[:], in_=permt_ext[:])
            mask_sb = constp.tile([128, 384], bf16, tag="mask")
            nc.sync.dma_start(out=mask_sb[:], in_=mask_ext[:])
            nc.gpsimd.memset(vones[:, :, HD : HD + 1], 1.0)

            # ACT is the bottleneck engine (exp); keep all evacs on DVE/Pool.
            def evac_dve(dst_ap, src_ap):
                nc.vector.tensor_copy(dst_ap, src_ap)

            def k_half(h2):
                # K^T: d-major, 2 accum groups of 512 cols in one psum tile
                pk = pbig.tile([128, 1024], f32, tag="big", name=f"pk{h2}")
                for q in range(2):
                    w = 1024 * h2 + 512 * q
                    for ec in range(EC):
                        nc.tensor.matmul(
                            pk[:, 512 * q : 512 * (q + 1)],
                            wk_sb[:, ec, :],
                            xt[:, ec, w : w + 512],
                            start=(ec == 0),
                            stop=(ec == EC - 1),
                        )
                evac_dve(kt_un[:, 1024 * h2 : 1024 * (h2 + 1)], pk[:])

            def v_half(h2):
                # V rows: stationary = xt chunk, 8 j-block groups per tile
                pv = pbig.tile([128, 1024], f32, tag="big", name=f"pv{h2}")
                for m in range(8):
                    lb = 8 * h2 + m
                    for ec in range(EC):
                        nc.tensor.matmul(
                            pv[:, 128 * m : 128 * (m + 1)],
                            xt[:, ec, 128 * lb : 128 * (lb + 1)],
                            wv_sb[:, ec, :],
                            start=(ec == 0),
                            stop=(ec == EC - 1),
                            skip_group_check=True,
                        )
                    if m % 4 == 3:
                        evac_dve(
                            vones[:, lb - 3 : lb + 1, 0:HD],
                            pv[:, 128 * (m - 3) : 128 * (m + 1)],
                        )

            def rope_units(src_un, dst, rot_sb):
                # dst = src*Rc + (PermT.T @ src)*Rs, bf16 d-major, as
                # independently emittable units (pipeline fill work).
                def mul1():
                    # Pool: dst = src * Rc (independent of the perm matmul)
                    nc.gpsimd.tensor_tensor(dst[:], src_un[:], rct_sb[:], op=MUL)

                def permch(ch):
                    def f():
                        rp = pbig.tile([128, 1024], f32, tag="big", name="rp")
                        for q in range(2):
                            sl = slice(
                                1024 * ch + 512 * q, 1024 * ch + 512 * (q + 1)
                            )
                            nc.tensor.matmul(
                                rp[:, 512 * q : 512 * (q + 1)],
                                permt_sb[:],
                                src_un[:, sl],
                                start=True,
                                stop=True,
                            )
                        # fused evac: rot = psum * Rs (DVE)
                        nc.vector.tensor_tensor(
                            rot_sb[:, 1024 * ch : 1024 * (ch + 1)],
                            rp[:],
                            rst_sb[:, 1024 * ch : 1024 * (ch + 1)],
                            op=MUL,
                        )
                    return f

                def add():
                    nc.vector.tensor_tensor(dst[:], dst[:], rot_sb[:], op=ADD)

                return [mul1, permch(0), permch(1), add]

            # ---- Q projection emission units (also used as pipeline fill) --
            def q_proj_units(hl, qun_tile):
                units = []
                for ch in range(2):
                    def mk(ch=ch):
                        pk = pbig.tile(
                            [128, 1024], f32, tag="big", name=f"pq{hl}_{ch}"
                        )
                        for q in range(2):
                            w = 1024 * ch + 512 * q
                            for ec in range(EC):
                                nc.tensor.matmul(
                                    pk[:, 512 * q : 512 * (q + 1)],
                                    wq_sb[:, ec, 128 * hl : 128 * (hl + 1)],
                                    xt[:, ec, w : w + 512],
                                    start=(ec == 0),
                                    stop=(ec == EC - 1),
                                )
                        evac_dve(
                            qun_tile[:, 1024 * ch : 1024 * (ch + 1)], pk[:]
                        )
                    units.append(mk)
                return units

            # ---- prefix: K/V/Q0 interleaved with the xT transposes ----
            kt_un = unp.tile([128, L], bf16, tag="un")
            qun = unp.tile([128, L], bf16, tag="un", name="qun0")
            q0_units = q_proj_units(0, qun)
            k_half(0)
            v_half(0)
            q0_units[0]()
            k_half(1)
            v_half(1)
            q0_units[1]()
            krot = rotp.tile([128, L], bf16, tag="rot", name="krot")
            for u in rope_units(kt_un, kt, krot):
                u()

            # ---- attention per head with lookahead-2 pipeline ----
            LOOKAHEAD = 2

            def attention(hl, qt_t, fill_units):
                # tp list: (g, t0, nblocks, is_first, is_last)
                tps = []
                for g in range(NG):
                    n_t = 2 * g + 2
                    for t0 in range(0, n_t, 4):
                        nb = min(4, n_t - t0)
                        tps.append((g, t0, nb, t0 == 0, t0 + nb == n_t))
                n = len(tps)
                sc_tiles = [None] * n
                pt_tiles = [None] * n
                pout_tiles = {}
                fill = list(fill_units)
                fill_start = max(0, n - 2 - len(fill))

                def emit_sc(i):
                    g, t0, nb, _, is_last = tps[i]
                    n_t = 2 * g + 2
                    sc = pbig.tile([128, 1024], f32, tag="big", name=f"sc{i}")
                    for s in range(nb):
                        t = t0 + s
                        if t == n_t - 1:
                            # odd diagonal block: half0 rows are fully
                            # masked; compute only the live 128 i-columns
                            nc.tensor.matmul(
                                sc[:, 256 * s : 256 * s + 128],
                                kt[:, 128 * t : 128 * (t + 1)],
                                qt_t[:, 256 * g + 128 : 256 * (g + 1)],
                                start=True,
                                stop=True,
                            )
                        else:
                            nc.tensor.matmul(
                                sc[:, 256 * s : 256 * (s + 1)],
                                kt[:, 128 * t : 128 * (t + 1)],
                                qt_t[:, 256 * g : 256 * (g + 1)],
                                start=True,
                                stop=True,
                            )
                    sc_tiles[i] = sc

                for i in range(-LOOKAHEAD, n):
                    j = i + LOOKAHEAD
                    if j < n:
                        emit_sc(j)
                    if i >= fill_start and fill:
                        fill.pop(0)()
                    if i < 0:
                        continue
                    g, t0, nb, is_first, is_last = tps[i]
                    w = 256 * nb - (128 if is_last else 0)
                    pt_t = ptp.tile([128, 1024], bf16, tag="pt")
                    nc.scalar.activation(
                        pt_t[:, 0:w], sc_tiles[i][:, 0:w], EXP, scale=SCALE
                    )
                    if is_last:
                        # mask the two diagonal j-blocks (last 384 used cols)
                        nc.vector.tensor_tensor(
                            pt_t[:, w - 384 : w],
                            pt_t[:, w - 384 : w],
                            mask_sb[:],
                            op=MUL,
                        )
                    if DEBUG and hl == 0 and i == 0:
                        nc.sync.dma_start(out=dbg_mask_ext[:], in_=mask_sb[:])
                        nc.sync.dma_start(
                            out=dbg_pt_ext[:, 0:w], in_=pt_t[:, 0:w]
                        )
                    if DEBUG and hl == 0 and i == 1:
                        nc.sync.dma_start(
                            out=dbg_pt1_ext[:, 0:w], in_=pt_t[:, 0:w]
                        )
                    pt_tiles[i] = pt_t
                    sc_tiles[i] = None
                    if is_first:
                        pout_tiles[g] = (
                            poutpA.tile(
                                [128, 512], f32, tag="poA", name=f"poA_{hl}_{g}"
                            ),
                            poutpB.tile(
                                [128, 512], f32, tag="poB", name=f"poB_{hl}_{g}"
                            ),
                        )
                    po = pout_tiles[g]
                    n_t = 2 * g + 2
                    for s in range(nb):
                        t = t0 + s
                        if t == n_t - 1:
                            # odd diagonal block: packed live half1 only
                            nc.tensor.matmul(
                                po[1][:, 0 : HD + 1],
                                pt_t[:, 256 * s : 256 * s + 128],
                                vones[:, t, :],
                                start=(t == 0),
                                stop=True,
                                skip_group_check=True,
                            )
                            continue
                        for half in range(2):
                            nc.tensor.matmul(
                                po[half][:, 0 : HD + 1],
                                pt_t[:, 256 * s + 128 * half : 256 * s + 128 * (half + 1)],
                                vones[:, t, :],
                                start=(t == 0),
                                stop=(t == n_t - 1 - (1 - half) and t != n_t - 1),
                                skip_group_check=True,
                            )
                    pt_tiles[i] = None
                    if is_last:
                        ob = ostp.tile([128, 2 * (HD + 1)], f32, tag="ob")
                        evac_dve(ob[:, 0 : HD + 1], po[0][:, 0 : HD + 1])
                        evac_dve(ob[:, HD + 1 : 2 * (HD + 1)], po[1][:, 0 : HD + 1])
                        if DEBUG and hl == 0 and g < 2:
                            nc.sync.dma_start(
                                out=dbg_ob_ext[g, :, :], in_=ob[:]
                            )
                        for half in range(2):
                            row0 = L * hl + 256 * g + 128 * half
                            nc.sync.dma_start(
                                out=out_ext[row0 : row0 + 128, :],
                                in_=ob[:, 129 * half : 129 * (half + 1)],
                            )
                        del pout_tiles[g]
                # leftover fill units (next head's remaining proj work)
                for u in fill:
                    u()

            # head 0 rope (prefix)
            qt_cur = qtp.tile([128, L], bf16, tag="qt", name="qt0")
            qrot = rotp.tile([128, L], bf16, tag="rot", name="qrot0")
            for u in rope_units(qun, qt_cur, qrot):
                u()

            for hl in range(HPC):
                if hl + 1 < HPC:
                    qun_next = unp.tile(
                        [128, L], bf16, tag="un", name=f"qun{hl + 1}"
                    )
                    qt_next = qtp.tile(
                        [128, L], bf16, tag="qt", name=f"qt{hl + 1}"
                    )
                    qrot_next = rotp.tile(
                        [128, L], bf16, tag="rot", name=f"qrot{hl + 1}"
                    )
                    fill_units = q_proj_units(hl + 1, qun_next) + rope_units(
                        qun_next, qt_next, qrot_next
                    )
                else:
                    qt_next = None
                    fill_units = []
                attention(hl, qt_cur, fill_units)
                qt_cur = qt_next
    return nc


def _get_program():
    if "nc" not in _CACHE:
        _ensure_ntff_hook()
        _CACHE["nc"] = _build_program()
    return _CACHE["nc"]


def kernel(x, Wq, Wk, Wv, _trace=False):
    _ensure_ntff_hook()
    from concourse.bass_utils import run_bass_kernel_spmd

    nc = _get_program()
    rct, rst, permt, mask = _host_tables()
    xb = [
        np.ascontiguousarray(np.asarray(x[b]).astype(BF16)) for b in range(B)
    ]
    wq_bf = np.asarray(Wq).astype(BF16)
    wk_bf = np.ascontiguousarray(np.asarray(Wk).astype(BF16))
    wv_bf = np.ascontiguousarray(np.asarray(Wv).astype(BF16))
    in_maps = []
    for c in range(N_CORES):
        b, hq = divmod(c, HPC)
        in_maps.append(
            {
                "x": xb[b],
                "wq": np.ascontiguousarray(
                    wq_bf[:, HPC * HD * hq : HPC * HD * (hq + 1)]
                ),
                "wk": wk_bf,
                "wv": wv_bf,
                "rct": rct,
                "rst": rst,
                "permt": permt,
                "mask": mask,
            }
        )
    res = run_bass_kernel_spmd(
        nc, in_maps, list(range(N_CORES)), trace=_trace
    )
    out = np.empty((B, L, NH * HD), np.float32)
    for c in range(N_CORES):
        b, hq = divmod(c, HPC)
        raw = res.results[c]["out"].reshape(HPC, L, HD + 1)
        vals = raw[:, :, :HD] / raw[:, :, HD : HD + 1]  # (4, L, 128)
        out[b, :, HPC * HD * hq : HPC * HD * (hq + 1)] = (
            vals.transpose(1, 0, 2).reshape(L, HPC * HD)
        )
    if _trace:
        return out, res
    return out


# revision 32
# speedup vs baseline: 1.0079x; 1.0079x over previous
"""Trainium2 Bass kernel for GroupRopeAttention (MQA + RoPE, causal).

Shapes (hardcoded): x (2, 2048, 1024), Wq (1024, 2048) -> 16 heads x 128,
Wk/Wv (1024, 128) single shared K/V head. Output (2, 2048, 2048).

Sharding: core c handles batch c//4 and query heads 4*(c%4)..4*(c%4)+3.
K/V are recomputed per core (no collectives). Each core returns a raw
(4*2048, 129) f32 slab = unnormalized PV output plus the softmax
denominator column; the host divides and reassembles.

Per-core pipeline (one TileContext, everything bf16 except PSUM):
  - xT (e-major x) via 16 hardware XBAR DMA-transposes (no PE transposes)
  - K^T projection d-major + RoPE (rotate-half = constant PermT matmul on
    PE; cos/sin tables are host-precomputed bf16 inputs)
  - V rows computed directly (xt-chunk stationary x Wv moving) into
    [V | ones] j-major slabs
  - per head: Q^T projection + RoPE, then causal attention with a
    lookahead-2 software pipeline: scores (kt_t stationary, 256-col qt
    movings) -> exp on ACT (scale folded) -> causal mask via one DVE
    multiply with a constant [tril | ones | shifted-tril] bf16 tile ->
    PV (pt stationary, [V|1] moving) accumulating output + denominator
    in PSUM, DMA'd raw to DRAM. Q-proj matmuls of the next head fill the
    PE pipeline-drain slots.
"""

import sys
import types

sys.path.insert(0, "/opt/trn_rl_repo")

import numpy as np
import ml_dtypes

BF16 = ml_dtypes.bfloat16

B, L, E = 2, 2048, 1024
NH, HD = 16, 128
N_CORES = 8
HPC = 4          # heads per core
THETA = 10000.0
SCALE = 1.0 / float(np.sqrt(HD))
EC = E // 128    # 8 e-chunks
NJ = L // 128    # 16 j-blocks
NG = L // 256    # 8 i-groups per head

_CACHE = {}


def _ensure_ntff_hook():
    """Register the NTFF profile hook if the image's antenv lacks it."""
    try:
        from antenv.axon_hooks import get_axon_ntff_profile_hook  # noqa: F401
        return
    except ImportError:
        pass
    import antenv

    mod = types.ModuleType("antenv.axon_hooks")
    mod._hook = None

    def set_axon_ntff_profile_hook(h):
        mod._hook = h

    def get_axon_ntff_profile_hook():
        return mod._hook

    mod.set_axon_ntff_profile_hook = set_axon_ntff_profile_hook
    mod.get_axon_ntff_profile_hook = get_axon_ntff_profile_hook
    sys.modules["antenv.axon_hooks"] = mod
    antenv.axon_hooks = mod
    try:
        from trn_agent_boot.trn_boot import _ntff_profile_via_ctypes

        set_axon_ntff_profile_hook(
            _ntff_profile_via_ctypes("/opt/axon/libaxon_pjrt.so")
        )
    except Exception:
        pass


def _host_tables():
    freqs = 1.0 / THETA ** (np.arange(0, HD, 2, dtype=np.float64) / HD)  # (64,)
    t = np.arange(L, dtype=np.float64)
    f = t[:, None] * freqs[None, :]  # (L, 64)
    f = np.repeat(f, 2, axis=-1)  # (L, 128)
    rct = np.ascontiguousarray(np.cos(f).T.astype(BF16))  # (128, L)
    rst = np.ascontiguousarray(np.sin(f).T.astype(BF16))  # (128, L)
    # rot[d] = -src[d+1] for even d, +src[d-1] for odd d, via rot = PermT.T @ src
    permt = np.zeros((HD, HD), dtype=BF16)
    for k in range(HD // 2):
        permt[2 * k, 2 * k + 1] = BF16(1.0)
        permt[2 * k + 1, 2 * k] = BF16(-1.0)
    # causal mask window for the two diagonal j-blocks of an i-group:
    # cols 0:128 -> t==2g block, i-local 0..127: keep i >= j
    # cols 128:256 -> t==2g block, i-local 128..255: always kept
    # cols 256:512 -> t==2g+1 block, i-local 0..255: keep i-128 >= j
    j = np.arange(128)[:, None]
    m1 = (np.arange(128)[None, :] >= j)
    mask = np.concatenate(
        [m1, np.ones((128, 128), bool), m1], axis=1
    ).astype(BF16)  # (128, 384)
    return rct, rst, permt, mask


def _build_program():
    import concourse.bass as bass
    import concourse.mybir as mybir
    import concourse.tile as tile
    from concourse.vector_clock import ScopedClock

    MAX_DRAIN_WAITS = 1

    def _max_inst_waits(inst):
        return 1

    class PatchedTileContext(tile.TileContext):
        # This walrus build rejects >2 sync waits per instruction. After
        # scheduling, hoist excess waits onto preceding nops on the same
        # engine (engines execute in order, so semantics are identical).
        def schedule_and_allocate(self, validate_deps=False):
            ret = super().schedule_and_allocate(validate_deps=validate_deps)
            for blk in self.nc.m.functions[0].blocks:
                new_insts = []
                for inst in blk.instructions:
                    mw = _max_inst_waits(inst)
                    si = inst.sync_info
                    waits = list(si.on_wait) if si and si.on_wait else []
                    if len(waits) > mw:
                        n_extra = len(waits) - mw
                        for i in range(0, n_extra, mw):
                            nop = mybir.InstNoOp(
                                name=self.nc.get_next_instruction_name(),
                                ins=[],
                                outs=[],
                            )
                            nop.engine = inst.engine
                            nop.sync_info = mybir.SyncInfo(
                                on_wait=waits[i : min(i + mw, n_extra)],
                                on_update=[],
                            )
                            self.nc.register_instruction(nop, overwrite=True)
                            new_insts.append(nop)
                        inst.sync_info = mybir.SyncInfo(
                            on_wait=waits[n_extra:],
                            on_update=list(si.on_update or []),
                        )
                    new_insts.append(inst)
                blk.instructions = new_insts
            return ret

        # The tile-exit drain gets the same treatment but must stay last in
        # its engine stream, so split it during emission instead.
        def _drain_and_barrier(self, tick_clock, wait_clock):
            drain_inst = self.nc.sync.drain()
            wait_clock.add_sem_waits(
                drain_inst.ins, ScopedClock({None: tick_clock.global_clock})
            )
            si = drain_inst.ins.sync_info
            waits = list(si.on_wait) if si and si.on_wait else []
            if len(waits) > MAX_DRAIN_WAITS:
                drain_inst.ins.sync_info = mybir.SyncInfo(
                    on_wait=waits[:MAX_DRAIN_WAITS],
                    on_update=list(si.on_update or []),
                )
                for i in range(MAX_DRAIN_WAITS, len(waits), MAX_DRAIN_WAITS):
                    nop = self.nc.sync.nop()
                    nop.ins.sync_info = mybir.SyncInfo(
                        on_wait=waits[i : i + MAX_DRAIN_WAITS], on_update=[]
                    )
            self.nc.all_engine_barrier()
            assert self.sems is not None
            popped = self.nc._tile_sem_poison_stack.pop()
            assert popped is self._sem_poison
            self.nc.clear_and_free_semaphores(
                list(self.sems.allocated().values())
            )
            self.nc.all_engine_barrier()

    f32 = mybir.dt.float32
    bf16 = mybir.dt.bfloat16
    EXP = mybir.ActivationFunctionType.Exp
    MUL = mybir.AluOpType.mult
    ADD = mybir.AluOpType.add

    nc = bass.Bass("TRN2", num_devices=N_CORES)

    x_ext = nc.declare_dram_parameter("x", [L, E], bf16, isOutput=False)
    wq_ext = nc.declare_dram_parameter("wq", [E, HPC * HD], bf16, isOutput=False)
    wk_ext = nc.declare_dram_parameter("wk", [E, HD], bf16, isOutput=False)
    wv_ext = nc.declare_dram_parameter("wv", [E, HD], bf16, isOutput=False)
    rct_ext = nc.declare_dram_parameter("rct", [HD, L], bf16, isOutput=False)
    rst_ext = nc.declare_dram_parameter("rst", [HD, L], bf16, isOutput=False)
    permt_ext = nc.declare_dram_parameter("permt", [HD, HD], bf16, isOutput=False)
    mask_ext = nc.declare_dram_parameter("mask", [128, 384], bf16, isOutput=False)
    out_ext = nc.declare_dram_parameter("out", [HPC * L, HD + 1], f32, isOutput=True)
    import os
    DEBUG = bool(os.environ.get("KERNEL_DEBUG"))
    if DEBUG:
        dbg_mask_ext = nc.declare_dram_parameter(
            "dbg_mask", [128, 512], bf16, isOutput=True
        )
        dbg_pt_ext = nc.declare_dram_parameter(
            "dbg_pt", [128, 1024], bf16, isOutput=True
        )
        dbg_pt1_ext = nc.declare_dram_parameter(
            "dbg_pt1", [128, 1024], bf16, isOutput=True
        )
        dbg_ob_ext = nc.declare_dram_parameter(
            "dbg_ob", [2, 128, 258], f32, isOutput=True
        )

    with PatchedTileContext(nc) as tc:
        with (
            tc.tile_pool(name="const", bufs=1) as constp,
            tc.tile_pool(name="un", bufs=2) as unp,
            tc.tile_pool(name="rot", bufs=2) as rotp,
            tc.tile_pool(name="qt", bufs=2) as qtp,
            tc.tile_pool(name="pt", bufs=4) as ptp,
            tc.tile_pool(name="ost", bufs=3) as ostp,
            tc.tile_pool(name="pbig", bufs=3, space="PSUM") as pbig,
            tc.tile_pool(name="poutA", bufs=1, space="PSUM") as poutpA,
            tc.tile_pool(name="poutB", bufs=1, space="PSUM") as poutpB,
        ):
            # ---- constants: wk/wv first so xT transposes start early ----
            wk_sb = constp.tile([128, EC, HD], bf16, tag="wk")
            nc.sync.dma_start(
                out=wk_sb[:], in_=wk_ext.rearrange("(c p) d -> p c d", p=128)
            )
            wv_sb = constp.tile([128, EC, HD], bf16, tag="wv")
            nc.sync.dma_start(
                out=wv_sb[:], in_=wv_ext.rearrange("(c p) d -> p c d", p=128)
            )

            xt = constp.tile([128, EC, L], bf16, tag="xt")
            vones = constp.tile([128, NJ, HD + 1], bf16, tag="vones")
            kt = constp.tile([128, L], bf16, tag="kt")

            # ---- xT via hardware DMA transpose (per quarter, per e-chunk),
            # alternating between the two HWDGE queues (SP and ACT) ----
            def emit_transposes(h2):
                for ec in range(EC):
                    nc.sync.dma_start(
                        out=xt[:, ec, 1024 * h2 : 1024 * (h2 + 1)],
                        in_=x_ext[
                            1024 * h2 : 1024 * (h2 + 1),
                            128 * ec : 128 * (ec + 1),
                        ],
                        transpose=True,
                    )

            emit_transposes(0)
            wq_sb = constp.tile([128, EC, HPC * HD], bf16, tag="wq")
            nc.sync.dma_start(
                out=wq_sb[:], in_=wq_ext.rearrange("(c p) d -> p c d", p=128)
            )
            emit_transposes(1)
            rct_sb = constp.tile([128, L], bf16, tag="rct")
            nc.sync.dma_start(out=rct_sb[:], in_=rct_ext[:])
            rst_sb = constp.tile([128, L], bf16, tag="rst")
            nc.sync.dma_start(out=rst_sb[:], in_=rst_ext[:])
            permt_sb = constp.tile([128, 128], bf16, tag="permt")
            nc.sync.dma_start(out=permt_sb[:], in_=permt_ext[:])
            mask_sb = constp.tile([128, 384], bf16, tag="mask")
            nc.sync.dma_start(out=mask_sb[:], in_=mask_ext[:])
            nc.gpsimd.memset(vones[:, :, HD : HD + 1], 1.0)

            # ACT is the bottleneck engine (exp); keep all evacs on DVE/Pool.
            def evac_dve(dst_ap, src_ap):
                nc.vector.tensor_copy(dst_ap, src_ap)

            def k_half(h2):
                # K^T: d-major, 2 accum groups of 512 cols in one psum tile
                pk = pbig.tile([128, 1024], f32, tag="big", name=f"pk{h2}")
                for q in range(2):
                    w = 1024 * h2 + 512 * q
                    for ec in range(EC):
                        nc.tensor.matmul(
                            pk[:, 512 * q : 512 * (q + 1)],
                            wk_sb[:, ec, :],
                            xt[:, ec, w : w + 512],
                            start=(ec == 0),
                            stop=(ec == EC - 1),
                        )
                evac_dve(kt_un[:, 1024 * h2 : 1024 * (h2 + 1)], pk[:])

            def v_half(h2):
                # V rows: stationary = xt chunk, 8 j-block groups per tile
                pv = pbig.tile([128, 1024], f32, tag="big", name=f"pv{h2}")
                for m in range(8):
                    lb = 8 * h2 + m
                    for ec in range(EC):
                        nc.tensor.matmul(
                            pv[:, 128 * m : 128 * (m + 1)],
                            xt[:, ec, 128 * lb : 128 * (lb + 1)],
                            wv_sb[:, ec, :],
                            start=(ec == 0),
                            stop=(ec == EC - 1),
                            skip_group_check=True,
                        )
                    if m % 4 == 3:
                        evac_dve(
                            vones[:, lb - 3 : lb + 1, 0:HD],
                            pv[:, 128 * (m - 3) : 128 * (m + 1)],
                        )

            def rope_units(src_un, dst, rot_sb):
                # dst = src*Rc + (PermT.T @ src)*Rs, bf16 d-major, as
                # independently emittable units (pipeline fill work).
                def mul1():
                    # Pool: dst = src * Rc (independent of the perm matmul)
                    nc.gpsimd.tensor_tensor(dst[:], src_un[:], rct_sb[:], op=MUL)

                def permch(ch):
                    def f():
                        rp = pbig.tile([128, 1024], f32, tag="big", name="rp")
                        for q in range(2):
                            sl = slice(
                                1024 * ch + 512 * q, 1024 * ch + 512 * (q + 1)
                            )
                            nc.tensor.matmul(
                                rp[:, 512 * q : 512 * (q + 1)],
                                permt_sb[:],
                                src_un[:, sl],
                                start=True,
                                stop=True,
                            )
                        # fused evac: rot = psum * Rs (DVE)
                        nc.vector.tensor_tensor(
                            rot_sb[:, 1024 * ch : 1024 * (ch + 1)],
                            rp[:],
                            rst_sb[:, 1024 * ch : 1024 * (ch + 1)],
                            op=MUL,
                        )
                    return f

                def add():
                    nc.vector.tensor_tensor(dst[:], dst[:], rot_sb[:], op=ADD)

                return [mul1, permch(0), permch(1), add]

            # ---- Q projection emission units (also used as pipeline fill) --
            def q_proj_units(hl, qun_tile):
                units = []
                for ch in range(2):
                    def mk(ch=ch):
                        pk = pbig.tile(
                            [128, 1024], f32, tag="big", name=f"pq{hl}_{ch}"
                        )
                        for q in range(2):
                            w = 1024 * ch + 512 * q
                            for ec in range(EC):
                                nc.tensor.matmul(
                                    pk[:, 512 * q : 512 * (q + 1)],
                                    wq_sb[:, ec, 128 * hl : 128 * (hl + 1)],
                                    xt[:, ec, w : w + 512],
                                    start=(ec == 0),
                                    stop=(ec == EC - 1),
                                )
                        evac_dve(
                            qun_tile[:, 1024 * ch : 1024 * (ch + 1)], pk[:]
                        )
                    units.append(mk)
                return units

            # ---- prefix: K/V/Q0 interleaved with the xT transposes ----
            kt_un = unp.tile([128, L], bf16, tag="un")
            qun = unp.tile([128, L], bf16, tag="un", name="qun0")
            q0_units = q_proj_units(0, qun)
            k_half(0)
            v_half(0)
            q0_units[0]()
            k_half(1)
            v_half(1)
            q0_units[1]()
            krot = rotp.tile([128, L], bf16, tag="rot", name="krot")
            for u in rope_units(kt_un, kt, krot):
                u()

            # ---- attention per head with lookahead-2 pipeline ----
            LOOKAHEAD = 3

            def attention(hl, qt_t, fill_units):
                # tp list: (g, t0, nblocks, is_first, is_last)
                tps = []
                for g in range(NG):
                    n_t = 2 * g + 2
                    for t0 in range(0, n_t, 4):
                        nb = min(4, n_t - t0)
                        tps.append((g, t0, nb, t0 == 0, t0 + nb == n_t))
                n = len(tps)
                sc_tiles = [None] * n
                pt_tiles = [None] * n
                pout_tiles = {}
                fill = list(fill_units)
                fill_start = max(0, n - 2 - len(fill))

                def emit_sc(i):
                    g, t0, nb, _, is_last = tps[i]
                    n_t = 2 * g + 2
                    sc = pbig.tile([128, 1024], f32, tag="big", name=f"sc{i}")
                    for s in range(nb):
                        t = t0 + s
                        if t == n_t - 1:
                            # odd diagonal block: half0 rows are fully
                            # masked; compute only the live 128 i-columns
                            nc.tensor.matmul(
                                sc[:, 256 * s : 256 * s + 128],
                                kt[:, 128 * t : 128 * (t + 1)],
                                qt_t[:, 256 * g + 128 : 256 * (g + 1)],
                                start=True,
                                stop=True,
                            )
                        else:
                            nc.tensor.matmul(
                                sc[:, 256 * s : 256 * (s + 1)],
                                kt[:, 128 * t : 128 * (t + 1)],
                                qt_t[:, 256 * g : 256 * (g + 1)],
                                start=True,
                                stop=True,
                            )
                    sc_tiles[i] = sc

                for i in range(-LOOKAHEAD, n):
                    j = i + LOOKAHEAD
                    if j < n:
                        emit_sc(j)
                    if i >= fill_start and fill:
                        fill.pop(0)()
                    if i < 0:
                        continue
                    g, t0, nb, is_first, is_last = tps[i]
                    w = 256 * nb - (128 if is_last else 0)
                    pt_t = ptp.tile([128, 1024], bf16, tag="pt")
                    nc.scalar.activation(
                        pt_t[:, 0:w], sc_tiles[i][:, 0:w], EXP, scale=SCALE
                    )
                    if is_last:
                        # mask the two diagonal j-blocks (last 384 used cols)
                        nc.vector.tensor_tensor(
                            pt_t[:, w - 384 : w],
                            pt_t[:, w - 384 : w],
                            mask_sb[:],
                            op=MUL,
                        )
                    if DEBUG and hl == 0 and i == 0:
                        nc.sync.dma_start(out=dbg_mask_ext[:], in_=mask_sb[:])
                        nc.sync.dma_start(
                            out=dbg_pt_ext[:, 0:w], in_=pt_t[:, 0:w]
                        )
                    if DEBUG and hl == 0 and i == 1:
                        nc.sync.dma_start(
                            out=dbg_pt1_ext[:, 0:w], in_=pt_t[:, 0:w]
                        )
                    pt_tiles[i] = pt_t
                    sc_tiles[i] = None
                    if is_first:
                        pout_tiles[g] = (
                            poutpA.tile(
                                [128, 512], f32, tag="poA", name=f"poA_{hl}_{g}"
                            ),
                            poutpB.tile(
                                [128, 512], f32, tag="poB", name=f"poB_{hl}_{g}"
                            ),
                        )
                    po = pout_tiles[g]
                    n_t = 2 * g + 2
                    for s in range(nb):
                        t = t0 + s
                        if t == n_t - 1:
                            # odd diagonal block: packed live half1 only
                            nc.tensor.matmul(
                                po[1][:, 0 : HD + 1],
                                pt_t[:, 256 * s : 256 * s + 128],
                                vones[:, t, :],
                                start=(t == 0),
                                stop=True,
                                skip_group_check=True,
                            )
                            continue
                        for half in range(2):
                            nc.tensor.matmul(
                                po[half][:, 0 : HD + 1],
                                pt_t[:, 256 * s + 128 * half : 256 * s + 128 * (half + 1)],
                                vones[:, t, :],
                                start=(t == 0),
                                stop=(t == n_t - 1 - (1 - half) and t != n_t - 1),
                                skip_group_check=True,
                            )
                    pt_tiles[i] = None
                    if is_last:
                        ob = ostp.tile([128, 2 * (HD + 1)], f32, tag="ob")
                        evac_dve(ob[:, 0 : HD + 1], po[0][:, 0 : HD + 1])
                        evac_dve(ob[:, HD + 1 : 2 * (HD + 1)], po[1][:, 0 : HD + 1])
                        if DEBUG and hl == 0 and g < 2:
                            nc.sync.dma_start(
                                out=dbg_ob_ext[g, :, :], in_=ob[:]
                            )
                        for half in range(2):
                            row0 = L * hl + 256 * g + 128 * half
                            nc.sync.dma_start(
                                out=out_ext[row0 : row0 + 128, :],
                                in_=ob[:, 129 * half : 129 * (half + 1)],
                            )
                        del pout_tiles[g]
                # leftover fill units (next head's remaining proj work)
                for u in fill:
                    u()

            # head 0 rope (prefix)
            qt_cur = qtp.tile([128, L], bf16, tag="qt", name="qt0")
            qrot = rotp.tile([128, L], bf16, tag="rot", name="qrot0")
            for u in rope_units(qun, qt_cur, qrot):
                u()

            for hl in range(HPC):
                if hl + 1 < HPC:
                    qun_next = unp.tile(
                        [128, L], bf16, tag="un", name=f"qun{hl + 1}"
                    )
                    qt_next = qtp.tile(
                        [128, L], bf16, tag="qt", name=f"qt{hl + 1}"
                    )
                    qrot_next = rotp.tile(
                        [128, L], bf16, tag="rot", name=f"qrot{hl + 1}"
                    )
                    fill_units = q_proj_units(hl + 1, qun_next) + rope_units(
                        qun_next, qt_next, qrot_next
                    )
                else:
                    qt_next = None
                    fill_units = []
                attention(hl, qt_cur, fill_units)
                qt_cur = qt_next
    return nc


def _get_program():
    if "nc" not in _CACHE:
        _ensure_ntff_hook()
        _CACHE["nc"] = _build_program()
    return _CACHE["nc"]


def kernel(x, Wq, Wk, Wv, _trace=False):
    _ensure_ntff_hook()
    from concourse.bass_utils import run_bass_kernel_spmd

    nc = _get_program()
    rct, rst, permt, mask = _host_tables()
    xb = [
        np.ascontiguousarray(np.asarray(x[b]).astype(BF16)) for b in range(B)
    ]
    wq_bf = np.asarray(Wq).astype(BF16)
    wk_bf = np.ascontiguousarray(np.asarray(Wk).astype(BF16))
    wv_bf = np.ascontiguousarray(np.asarray(Wv).astype(BF16))
    in_maps = []
    for c in range(N_CORES):
        b, hq = divmod(c, HPC)
        in_maps.append(
            {
                "x": xb[b],
                "wq": np.ascontiguousarray(
                    wq_bf[:, HPC * HD * hq : HPC * HD * (hq + 1)]
                ),
                "wk": wk_bf,
                "wv": wv_bf,
                "rct": rct,
                "rst": rst,
                "permt": permt,
                "mask": mask,
            }
        )
    res = run_bass_kernel_spmd(
        nc, in_maps, list(range(N_CORES)), trace=_trace
    )
    out = np.empty((B, L, NH * HD), np.float32)
    for c in range(N_CORES):
        b, hq = divmod(c, HPC)
        raw = res.results[c]["out"].reshape(HPC, L, HD + 1)
        vals = raw[:, :, :HD] / raw[:, :, HD : HD + 1]  # (4, L, 128)
        out[b, :, HPC * HD * hq : HPC * HD * (hq + 1)] = (
            vals.transpose(1, 0, 2).reshape(L, HPC * HD)
        )
    if _trace:
        return out, res
    return out


# revision 33
# speedup vs baseline: 1.0177x; 1.0097x over previous
"""Trainium2 Bass kernel for GroupRopeAttention (MQA + RoPE, causal).

Shapes (hardcoded): x (2, 2048, 1024), Wq (1024, 2048) -> 16 heads x 128,
Wk/Wv (1024, 128) single shared K/V head. Output (2, 2048, 2048).

Sharding: core c handles batch c//4 and query heads 4*(c%4)..4*(c%4)+3.
K/V are recomputed per core (no collectives). Each core returns a raw
(4*2048, 129) f32 slab = unnormalized PV output plus the softmax
denominator column; the host divides and reassembles.

Per-core pipeline (one TileContext, everything bf16 except PSUM):
  - xT (e-major x) via 16 hardware XBAR DMA-transposes (no PE transposes)
  - K^T projection d-major + RoPE (rotate-half = constant PermT matmul on
    PE; cos/sin tables are host-precomputed bf16 inputs)
  - V rows computed directly (xt-chunk stationary x Wv moving) into
    [V | ones] j-major slabs
  - per head: Q^T projection + RoPE, then causal attention with a
    lookahead-2 software pipeline: scores (kt_t stationary, 256-col qt
    movings) -> exp on ACT (scale folded) -> causal mask via one DVE
    multiply with a constant [tril | ones | shifted-tril] bf16 tile ->
    PV (pt stationary, [V|1] moving) accumulating output + denominator
    in PSUM, DMA'd raw to DRAM. Q-proj matmuls of the next head fill the
    PE pipeline-drain slots.
"""

import sys
import types

sys.path.insert(0, "/opt/trn_rl_repo")

import numpy as np
import ml_dtypes

BF16 = ml_dtypes.bfloat16

B, L, E = 2, 2048, 1024
NH, HD = 16, 128
N_CORES = 8
HPC = 4          # heads per core
THETA = 10000.0
SCALE = 1.0 / float(np.sqrt(HD))
EC = E // 128    # 8 e-chunks
NJ = L // 128    # 16 j-blocks
NG = L // 256    # 8 i-groups per head

_CACHE = {}


def _ensure_ntff_hook():
    """Register the NTFF profile hook if the image's antenv lacks it."""
    try:
        from antenv.axon_hooks import get_axon_ntff_profile_hook  # noqa: F401
        return
    except ImportError:
        pass
    import antenv

    mod = types.ModuleType("antenv.axon_hooks")
    mod._hook = None

    def set_axon_ntff_profile_hook(h):
        mod._hook = h

    def get_axon_ntff_profile_hook():
        return mod._hook

    mod.set_axon_ntff_profile_hook = set_axon_ntff_profile_hook
    mod.get_axon_ntff_profile_hook = get_axon_ntff_profile_hook
    sys.modules["antenv.axon_hooks"] = mod
    antenv.axon_hooks = mod
    try:
        from trn_agent_boot.trn_boot import _ntff_profile_via_ctypes

        set_axon_ntff_profile_hook(
            _ntff_profile_via_ctypes("/opt/axon/libaxon_pjrt.so")
        )
    except Exception:
        pass


def _host_tables():
    freqs = 1.0 / THETA ** (np.arange(0, HD, 2, dtype=np.float64) / HD)  # (64,)
    t = np.arange(L, dtype=np.float64)
    f = t[:, None] * freqs[None, :]  # (L, 64)
    f = np.repeat(f, 2, axis=-1)  # (L, 128)
    rct = np.ascontiguousarray(np.cos(f).T.astype(BF16))  # (128, L)
    rst = np.ascontiguousarray(np.sin(f).T.astype(BF16))  # (128, L)
    # rot[d] = -src[d+1] for even d, +src[d-1] for odd d, via rot = PermT.T @ src
    permt = np.zeros((HD, HD), dtype=BF16)
    for k in range(HD // 2):
        permt[2 * k, 2 * k + 1] = BF16(1.0)
        permt[2 * k + 1, 2 * k] = BF16(-1.0)
    # causal mask window for the two diagonal j-blocks of an i-group:
    # cols 0:128 -> t==2g block, i-local 0..127: keep i >= j
    # cols 128:256 -> t==2g block, i-local 128..255: always kept
    # cols 256:512 -> t==2g+1 block, i-local 0..255: keep i-128 >= j
    j = np.arange(128)[:, None]
    m1 = (np.arange(128)[None, :] >= j)
    mask = np.concatenate(
        [m1, np.ones((128, 128), bool), m1], axis=1
    ).astype(BF16)  # (128, 384)
    return rct, rst, permt, mask


def _build_program():
    import concourse.bass as bass
    import concourse.mybir as mybir
    import concourse.tile as tile
    from concourse.vector_clock import ScopedClock

    MAX_DRAIN_WAITS = 1

    def _max_inst_waits(inst):
        return 1

    class PatchedTileContext(tile.TileContext):
        # This walrus build rejects >2 sync waits per instruction. After
        # scheduling, hoist excess waits onto preceding nops on the same
        # engine (engines execute in order, so semantics are identical).
        def schedule_and_allocate(self, validate_deps=False):
            ret = super().schedule_and_allocate(validate_deps=validate_deps)
            for blk in self.nc.m.functions[0].blocks:
                new_insts = []
                for inst in blk.instructions:
                    mw = _max_inst_waits(inst)
                    si = inst.sync_info
                    waits = list(si.on_wait) if si and si.on_wait else []
                    if len(waits) > mw:
                        n_extra = len(waits) - mw
                        for i in range(0, n_extra, mw):
                            nop = mybir.InstNoOp(
                                name=self.nc.get_next_instruction_name(),
                                ins=[],
                                outs=[],
                            )
                            nop.engine = inst.engine
                            nop.sync_info = mybir.SyncInfo(
                                on_wait=waits[i : min(i + mw, n_extra)],
                                on_update=[],
                            )
                            self.nc.register_instruction(nop, overwrite=True)
                            new_insts.append(nop)
                        inst.sync_info = mybir.SyncInfo(
                            on_wait=waits[n_extra:],
                            on_update=list(si.on_update or []),
                        )
                    new_insts.append(inst)
                blk.instructions = new_insts
            return ret

        # The tile-exit drain gets the same treatment but must stay last in
        # its engine stream, so split it during emission instead.
        def _drain_and_barrier(self, tick_clock, wait_clock):
            drain_inst = self.nc.sync.drain()
            wait_clock.add_sem_waits(
                drain_inst.ins, ScopedClock({None: tick_clock.global_clock})
            )
            si = drain_inst.ins.sync_info
            waits = list(si.on_wait) if si and si.on_wait else []
            if len(waits) > MAX_DRAIN_WAITS:
                drain_inst.ins.sync_info = mybir.SyncInfo(
                    on_wait=waits[:MAX_DRAIN_WAITS],
                    on_update=list(si.on_update or []),
                )
                for i in range(MAX_DRAIN_WAITS, len(waits), MAX_DRAIN_WAITS):
                    nop = self.nc.sync.nop()
                    nop.ins.sync_info = mybir.SyncInfo(
                        on_wait=waits[i : i + MAX_DRAIN_WAITS], on_update=[]
                    )
            self.nc.all_engine_barrier()
            assert self.sems is not None
            popped = self.nc._tile_sem_poison_stack.pop()
            assert popped is self._sem_poison
            self.nc.clear_and_free_semaphores(
                list(self.sems.allocated().values())
            )
            self.nc.all_engine_barrier()

    f32 = mybir.dt.float32
    bf16 = mybir.dt.bfloat16
    EXP = mybir.ActivationFunctionType.Exp
    MUL = mybir.AluOpType.mult
    ADD = mybir.AluOpType.add

    nc = bass.Bass("TRN2", num_devices=N_CORES)

    x_ext = nc.declare_dram_parameter("x", [L, E], bf16, isOutput=False)
    wq_ext = nc.declare_dram_parameter("wq", [E, HPC * HD], bf16, isOutput=False)
    wk_ext = nc.declare_dram_parameter("wk", [E, HD], bf16, isOutput=False)
    wv_ext = nc.declare_dram_parameter("wv", [E, HD], bf16, isOutput=False)
    rct_ext = nc.declare_dram_parameter("rct", [HD, L], bf16, isOutput=False)
    rst_ext = nc.declare_dram_parameter("rst", [HD, L], bf16, isOutput=False)
    permt_ext = nc.declare_dram_parameter("permt", [HD, HD], bf16, isOutput=False)
    mask_ext = nc.declare_dram_parameter("mask", [128, 384], bf16, isOutput=False)
    out_ext = nc.declare_dram_parameter("out", [HPC * L, HD + 1], f32, isOutput=True)
    import os
    DEBUG = bool(os.environ.get("KERNEL_DEBUG"))
    if DEBUG:
        dbg_mask_ext = nc.declare_dram_parameter(
            "dbg_mask", [128, 512], bf16, isOutput=True
        )
        dbg_pt_ext = nc.declare_dram_parameter(
            "dbg_pt", [128, 1024], bf16, isOutput=True
        )
        dbg_pt1_ext = nc.declare_dram_parameter(
            "dbg_pt1", [128, 1024], bf16, isOutput=True
        )
        dbg_ob_ext = nc.declare_dram_parameter(
            "dbg_ob", [2, 128, 258], f32, isOutput=True
        )

    with PatchedTileContext(nc) as tc:
        with (
            tc.tile_pool(name="const", bufs=1) as constp,
            tc.tile_pool(name="un", bufs=2) as unp,
            tc.tile_pool(name="rot", bufs=2) as rotp,
            tc.tile_pool(name="qt", bufs=2) as qtp,
            tc.tile_pool(name="pt", bufs=4) as ptp,
            tc.tile_pool(name="ost", bufs=3) as ostp,
            tc.tile_pool(name="pbig", bufs=3, space="PSUM") as pbig,
            tc.tile_pool(name="poutA", bufs=1, space="PSUM") as poutpA,
            tc.tile_pool(name="poutB", bufs=1, space="PSUM") as poutpB,
        ):
            # ---- constants: wk/wv first so xT transposes start early ----
            wk_sb = constp.tile([128, EC, HD], bf16, tag="wk")
            nc.sync.dma_start(
                out=wk_sb[:], in_=wk_ext.rearrange("(c p) d -> p c d", p=128)
            )
            wv_sb = constp.tile([128, EC, HD], bf16, tag="wv")
            nc.sync.dma_start(
                out=wv_sb[:], in_=wv_ext.rearrange("(c p) d -> p c d", p=128)
            )

            xt = constp.tile([128, EC, L], bf16, tag="xt")
            vones = constp.tile([128, NJ, HD + 1], bf16, tag="vones")
            kt = constp.tile([128, L], bf16, tag="kt")

            # ---- xT via hardware DMA transpose (per quarter, per e-chunk),
            # alternating between the two HWDGE queues (SP and ACT) ----
            def emit_transposes(h2):
                for ec in range(EC):
                    nc.sync.dma_start(
                        out=xt[:, ec, 1024 * h2 : 1024 * (h2 + 1)],
                        in_=x_ext[
                            1024 * h2 : 1024 * (h2 + 1),
                            128 * ec : 128 * (ec + 1),
                        ],
                        transpose=True,
                    )

            emit_transposes(0)
            wq_sb = constp.tile([128, EC, HPC * HD], bf16, tag="wq")
            nc.sync.dma_start(
                out=wq_sb[:], in_=wq_ext.rearrange("(c p) d -> p c d", p=128)
            )
            emit_transposes(1)
            rct_sb = constp.tile([128, L], bf16, tag="rct")
            nc.sync.dma_start(out=rct_sb[:], in_=rct_ext[:])
            rst_sb = constp.tile([128, L], bf16, tag="rst")
            nc.sync.dma_start(out=rst_sb[:], in_=rst_ext[:])
            permt_sb = constp.tile([128, 128], bf16, tag="permt")
            nc.sync.dma_start(out=permt_sb[:], in_=permt_ext[:])
            mask_sb = constp.tile([128, 384], bf16, tag="mask")
            nc.sync.dma_start(out=mask_sb[:], in_=mask_ext[:])
            nc.gpsimd.memset(vones[:, :, HD : HD + 1], 1.0)

            # ACT is the bottleneck engine (exp); keep all evacs on DVE/Pool.
            def evac_dve(dst_ap, src_ap):
                nc.vector.tensor_copy(dst_ap, src_ap)

            def k_half(h2):
                # K^T: d-major, 2 accum groups of 512 cols in one psum tile
                pk = pbig.tile([128, 1024], f32, tag="big", name=f"pk{h2}")
                for q in range(2):
                    w = 1024 * h2 + 512 * q
                    for ec in range(EC):
                        nc.tensor.matmul(
                            pk[:, 512 * q : 512 * (q + 1)],
                            wk_sb[:, ec, :],
                            xt[:, ec, w : w + 512],
                            start=(ec == 0),
                            stop=(ec == EC - 1),
                        )
                evac_dve(kt_un[:, 1024 * h2 : 1024 * (h2 + 1)], pk[:])

            def v_half(h2):
                # V rows: stationary = xt chunk, 8 j-block groups per tile
                pv = pbig.tile([128, 1024], f32, tag="big", name=f"pv{h2}")
                for m in range(8):
                    lb = 8 * h2 + m
                    for ec in range(EC):
                        nc.tensor.matmul(
                            pv[:, 128 * m : 128 * (m + 1)],
                            xt[:, ec, 128 * lb : 128 * (lb + 1)],
                            wv_sb[:, ec, :],
                            start=(ec == 0),
                            stop=(ec == EC - 1),
                            skip_group_check=True,
                        )
                    if m % 4 == 3:
                        evac_dve(
                            vones[:, lb - 3 : lb + 1, 0:HD],
                            pv[:, 128 * (m - 3) : 128 * (m + 1)],
                        )

            def rope_units(src_un, dst, rot_sb):
                # dst = src*Rc + (PermT.T @ src)*Rs, bf16 d-major, as
                # independently emittable units (pipeline fill work).
                # Chunked per 1024 cols so the consumer's first i-groups
                # unblock after chunk 0's add.
                def mul1(ch):
                    def f():
                        sl = slice(1024 * ch, 1024 * (ch + 1))
                        # Pool: dst = src * Rc (independent of the perm)
                        nc.gpsimd.tensor_tensor(
                            dst[:, sl], src_un[:, sl], rct_sb[:, sl], op=MUL
                        )
                    return f

                def permch(ch):
                    def f():
                        rp = pbig.tile([128, 1024], f32, tag="big", name="rp")
                        for q in range(2):
                            sl = slice(
                                1024 * ch + 512 * q, 1024 * ch + 512 * (q + 1)
                            )
                            nc.tensor.matmul(
                                rp[:, 512 * q : 512 * (q + 1)],
                                permt_sb[:],
                                src_un[:, sl],
                                start=True,
                                stop=True,
                            )
                        # fused evac: rot = psum * Rs (DVE)
                        nc.vector.tensor_tensor(
                            rot_sb[:, 1024 * ch : 1024 * (ch + 1)],
                            rp[:],
                            rst_sb[:, 1024 * ch : 1024 * (ch + 1)],
                            op=MUL,
                        )
                    return f

                def add(ch):
                    def f():
                        sl = slice(1024 * ch, 1024 * (ch + 1))
                        nc.vector.tensor_tensor(
                            dst[:, sl], dst[:, sl], rot_sb[:, sl], op=ADD
                        )
                    return f

                return [mul1(0), permch(0), mul1(1), permch(1), add(0), add(1)]

            # ---- Q projection emission units (also used as pipeline fill) --
            def q_proj_units(hl, qun_tile):
                units = []
                for ch in range(2):
                    def mk(ch=ch):
                        pk = pbig.tile(
                            [128, 1024], f32, tag="big", name=f"pq{hl}_{ch}"
                        )
                        for q in range(2):
                            w = 1024 * ch + 512 * q
                            for ec in range(EC):
                                nc.tensor.matmul(
                                    pk[:, 512 * q : 512 * (q + 1)],
                                    wq_sb[:, ec, 128 * hl : 128 * (hl + 1)],
                                    xt[:, ec, w : w + 512],
                                    start=(ec == 0),
                                    stop=(ec == EC - 1),
                                )
                        evac_dve(
                            qun_tile[:, 1024 * ch : 1024 * (ch + 1)], pk[:]
                        )
                    units.append(mk)
                return units

            # ---- prefix: K/V/Q0 interleaved with the xT transposes ----
            kt_un = unp.tile([128, L], bf16, tag="un")
            qun = unp.tile([128, L], bf16, tag="un", name="qun0")
            q0_units = q_proj_units(0, qun)
            k_half(0)
            v_half(0)
            q0_units[0]()
            k_half(1)
            v_half(1)
            q0_units[1]()
            krot = rotp.tile([128, L], bf16, tag="rot", name="krot")
            for u in rope_units(kt_un, kt, krot):
                u()

            # ---- attention per head with lookahead-2 pipeline ----
            LOOKAHEAD = 3

            def attention(hl, qt_t, fill_units):
                # tp list: (g, t0, nblocks, is_first, is_last)
                tps = []
                for g in range(NG):
                    n_t = 2 * g + 2
                    for t0 in range(0, n_t, 4):
                        nb = min(4, n_t - t0)
                        tps.append((g, t0, nb, t0 == 0, t0 + nb == n_t))
                n = len(tps)
                sc_tiles = [None] * n
                pt_tiles = [None] * n
                pout_tiles = {}
                fill = list(fill_units)
                fill_start = max(0, n - 2 - len(fill))

                def emit_sc(i):
                    g, t0, nb, _, is_last = tps[i]
                    n_t = 2 * g + 2
                    sc = pbig.tile([128, 1024], f32, tag="big", name=f"sc{i}")
                    for s in range(nb):
                        t = t0 + s
                        if t == n_t - 1:
                            # odd diagonal block: half0 rows are fully
                            # masked; compute only the live 128 i-columns
                            nc.tensor.matmul(
                                sc[:, 256 * s : 256 * s + 128],
                                kt[:, 128 * t : 128 * (t + 1)],
                                qt_t[:, 256 * g + 128 : 256 * (g + 1)],
                                start=True,
                                stop=True,
                            )
                        else:
                            nc.tensor.matmul(
                                sc[:, 256 * s : 256 * (s + 1)],
                                kt[:, 128 * t : 128 * (t + 1)],
                                qt_t[:, 256 * g : 256 * (g + 1)],
                                start=True,
                                stop=True,
                            )
                    sc_tiles[i] = sc

                for i in range(-LOOKAHEAD, n):
                    j = i + LOOKAHEAD
                    if j < n:
                        emit_sc(j)
                    if i >= fill_start and fill:
                        fill.pop(0)()
                    if i < 0:
                        continue
                    g, t0, nb, is_first, is_last = tps[i]
                    w = 256 * nb - (128 if is_last else 0)
                    pt_t = ptp.tile([128, 1024], bf16, tag="pt")
                    nc.scalar.activation(
                        pt_t[:, 0:w], sc_tiles[i][:, 0:w], EXP, scale=SCALE
                    )
                    if is_last:
                        # mask the two diagonal j-blocks (last 384 used cols)
                        nc.vector.tensor_tensor(
                            pt_t[:, w - 384 : w],
                            pt_t[:, w - 384 : w],
                            mask_sb[:],
                            op=MUL,
                        )
                    if DEBUG and hl == 0 and i == 0:
                        nc.sync.dma_start(out=dbg_mask_ext[:], in_=mask_sb[:])
                        nc.sync.dma_start(
                            out=dbg_pt_ext[:, 0:w], in_=pt_t[:, 0:w]
                        )
                    if DEBUG and hl == 0 and i == 1:
                        nc.sync.dma_start(
                            out=dbg_pt1_ext[:, 0:w], in_=pt_t[:, 0:w]
                        )
                    pt_tiles[i] = pt_t
                    sc_tiles[i] = None
                    if is_first:
                        pout_tiles[g] = (
                            poutpA.tile(
                                [128, 512], f32, tag="poA", name=f"poA_{hl}_{g}"
                            ),
                            poutpB.tile(
                                [128, 512], f32, tag="poB", name=f"poB_{hl}_{g}"
                            ),
                        )
                    po = pout_tiles[g]
                    n_t = 2 * g + 2
                    for s in range(nb):
                        t = t0 + s
                        if t == n_t - 1:
                            # odd diagonal block: packed live half1 only
                            nc.tensor.matmul(
                                po[1][:, 0 : HD + 1],
                                pt_t[:, 256 * s : 256 * s + 128],
                                vones[:, t, :],
                                start=(t == 0),
                                stop=True,
                                skip_group_check=True,
                            )
                            continue
                        for half in range(2):
                            nc.tensor.matmul(
                                po[half][:, 0 : HD + 1],
                                pt_t[:, 256 * s + 128 * half : 256 * s + 128 * (half + 1)],
                                vones[:, t, :],
                                start=(t == 0),
                                stop=(t == n_t - 1 - (1 - half) and t != n_t - 1),
                                skip_group_check=True,
                            )
                    pt_tiles[i] = None
                    if is_last:
                        ob = ostp.tile([128, 2 * (HD + 1)], f32, tag="ob")
                        evac_dve(ob[:, 0 : HD + 1], po[0][:, 0 : HD + 1])
                        evac_dve(ob[:, HD + 1 : 2 * (HD + 1)], po[1][:, 0 : HD + 1])
                        if DEBUG and hl == 0 and g < 2:
                            nc.sync.dma_start(
                                out=dbg_ob_ext[g, :, :], in_=ob[:]
                            )
                        for half in range(2):
                            row0 = L * hl + 256 * g + 128 * half
                            nc.sync.dma_start(
                                out=out_ext[row0 : row0 + 128, :],
                                in_=ob[:, 129 * half : 129 * (half + 1)],
                            )
                        del pout_tiles[g]
                # leftover fill units (next head's remaining proj work)
                for u in fill:
                    u()

            # head 0 rope (prefix)
            qt_cur = qtp.tile([128, L], bf16, tag="qt", name="qt0")
            qrot = rotp.tile([128, L], bf16, tag="rot", name="qrot0")
            for u in rope_units(qun, qt_cur, qrot):
                u()

            for hl in range(HPC):
                if hl + 1 < HPC:
                    qun_next = unp.tile(
                        [128, L], bf16, tag="un", name=f"qun{hl + 1}"
                    )
                    qt_next = qtp.tile(
                        [128, L], bf16, tag="qt", name=f"qt{hl + 1}"
                    )
                    qrot_next = rotp.tile(
                        [128, L], bf16, tag="rot", name=f"qrot{hl + 1}"
                    )
                    fill_units = q_proj_units(hl + 1, qun_next) + rope_units(
                        qun_next, qt_next, qrot_next
                    )
                else:
                    qt_next = None
                    fill_units = []
                attention(hl, qt_cur, fill_units)
                qt_cur = qt_next
    return nc


def _get_program():
    if "nc" not in _CACHE:
        _ensure_ntff_hook()
        _CACHE["nc"] = _build_program()
    return _CACHE["nc"]


def kernel(x, Wq, Wk, Wv, _trace=False):
    _ensure_ntff_hook()
    from concourse.bass_utils import run_bass_kernel_spmd

    nc = _get_program()
    rct, rst, permt, mask = _host_tables()
    xb = [
        np.ascontiguousarray(np.asarray(x[b]).astype(BF16)) for b in range(B)
    ]
    wq_bf = np.asarray(Wq).astype(BF16)
    wk_bf = np.ascontiguousarray(np.asarray(Wk).astype(BF16))
    wv_bf = np.ascontiguousarray(np.asarray(Wv).astype(BF16))
    in_maps = []
    for c in range(N_CORES):
        b, hq = divmod(c, HPC)
        in_maps.append(
            {
                "x": xb[b],
                "wq": np.ascontiguousarray(
                    wq_bf[:, HPC * HD * hq : HPC * HD * (hq + 1)]
                ),
                "wk": wk_bf,
                "wv": wv_bf,
                "rct": rct,
                "rst": rst,
                "permt": permt,
                "mask": mask,
            }
        )
    res = run_bass_kernel_spmd(
        nc, in_maps, list(range(N_CORES)), trace=_trace
    )
    out = np.empty((B, L, NH * HD), np.float32)
    for c in range(N_CORES):
        b, hq = divmod(c, HPC)
        raw = res.results[c]["out"].reshape(HPC, L, HD + 1)
        vals = raw[:, :, :HD] / raw[:, :, HD : HD + 1]  # (4, L, 128)
        out[b, :, HPC * HD * hq : HPC * HD * (hq + 1)] = (
            vals.transpose(1, 0, 2).reshape(L, HPC * HD)
        )
    if _trace:
        return out, res
    return out


# revision 34
# speedup vs baseline: 1.0548x; 1.0365x over previous
"""Trainium2 Bass kernel for GroupRopeAttention (MQA + RoPE, causal).

Shapes (hardcoded): x (2, 2048, 1024), Wq (1024, 2048) -> 16 heads x 128,
Wk/Wv (1024, 128) single shared K/V head. Output (2, 2048, 2048).

Sharding: core c handles batch c//4 and query heads 4*(c%4)..4*(c%4)+3.
K/V are recomputed per core (no collectives). Each core returns a raw
(4*2048, 129) f32 slab = unnormalized PV output plus the softmax
denominator column; the host divides and reassembles.

Per-core pipeline (one TileContext, everything bf16 except PSUM):
  - xT (e-major x) via 16 hardware XBAR DMA-transposes (no PE transposes)
  - K^T projection d-major + RoPE (rotate-half = constant PermT matmul on
    PE; cos/sin tables are host-precomputed bf16 inputs)
  - V rows computed directly (xt-chunk stationary x Wv moving) into
    [V | ones] j-major slabs
  - per head: Q^T projection + RoPE, then causal attention with a
    lookahead-2 software pipeline: scores (kt_t stationary, 256-col qt
    movings) -> exp on ACT (scale folded) -> causal mask via one DVE
    multiply with a constant [tril | ones | shifted-tril] bf16 tile ->
    PV (pt stationary, [V|1] moving) accumulating output + denominator
    in PSUM, DMA'd raw to DRAM. Q-proj matmuls of the next head fill the
    PE pipeline-drain slots.
"""

import sys
import types

sys.path.insert(0, "/opt/trn_rl_repo")

import numpy as np
import ml_dtypes

BF16 = ml_dtypes.bfloat16

B, L, E = 2, 2048, 1024
NH, HD = 16, 128
N_CORES = 8
HPC = 4          # heads per core
THETA = 10000.0
SCALE = 1.0 / float(np.sqrt(HD))
EC = E // 128    # 8 e-chunks
NJ = L // 128    # 16 j-blocks
NG = L // 256    # 8 i-groups per head

_CACHE = {}


def _ensure_ntff_hook():
    """Register the NTFF profile hook if the image's antenv lacks it."""
    try:
        from antenv.axon_hooks import get_axon_ntff_profile_hook  # noqa: F401
        return
    except ImportError:
        pass
    import antenv

    mod = types.ModuleType("antenv.axon_hooks")
    mod._hook = None

    def set_axon_ntff_profile_hook(h):
        mod._hook = h

    def get_axon_ntff_profile_hook():
        return mod._hook

    mod.set_axon_ntff_profile_hook = set_axon_ntff_profile_hook
    mod.get_axon_ntff_profile_hook = get_axon_ntff_profile_hook
    sys.modules["antenv.axon_hooks"] = mod
    antenv.axon_hooks = mod
    try:
        from trn_agent_boot.trn_boot import _ntff_profile_via_ctypes

        set_axon_ntff_profile_hook(
            _ntff_profile_via_ctypes("/opt/axon/libaxon_pjrt.so")
        )
    except Exception:
        pass


def _host_tables():
    freqs = 1.0 / THETA ** (np.arange(0, HD, 2, dtype=np.float64) / HD)  # (64,)
    t = np.arange(L, dtype=np.float64)
    f = t[:, None] * freqs[None, :]  # (L, 64)
    f = np.repeat(f, 2, axis=-1)  # (L, 128)
    rct = np.ascontiguousarray(np.cos(f).T.astype(BF16))  # (128, L)
    rst = np.ascontiguousarray(np.sin(f).T.astype(BF16))  # (128, L)
    # rot[d] = -src[d+1] for even d, +src[d-1] for odd d, via rot = PermT.T @ src
    permt = np.zeros((HD, HD), dtype=BF16)
    for k in range(HD // 2):
        permt[2 * k, 2 * k + 1] = BF16(1.0)
        permt[2 * k + 1, 2 * k] = BF16(-1.0)
    # causal mask window for the two diagonal j-blocks of an i-group:
    # cols 0:128 -> t==2g block, i-local 0..127: keep i >= j
    # cols 128:256 -> t==2g block, i-local 128..255: always kept
    # cols 256:512 -> t==2g+1 block, i-local 0..255: keep i-128 >= j
    j = np.arange(128)[:, None]
    m1 = (np.arange(128)[None, :] >= j)
    mask = np.concatenate(
        [m1, np.ones((128, 128), bool), m1], axis=1
    ).astype(BF16)  # (128, 384)
    return rct, rst, permt, mask


def _build_program():
    import concourse.bass as bass
    import concourse.mybir as mybir
    import concourse.tile as tile
    from concourse.vector_clock import ScopedClock

    MAX_DRAIN_WAITS = 1

    def _max_inst_waits(inst):
        return 1

    class PatchedTileContext(tile.TileContext):
        # This walrus build rejects >2 sync waits per instruction. After
        # scheduling, hoist excess waits onto preceding nops on the same
        # engine (engines execute in order, so semantics are identical).
        def schedule_and_allocate(self, validate_deps=False):
            ret = super().schedule_and_allocate(validate_deps=validate_deps)
            for blk in self.nc.m.functions[0].blocks:
                new_insts = []
                for inst in blk.instructions:
                    mw = _max_inst_waits(inst)
                    si = inst.sync_info
                    waits = list(si.on_wait) if si and si.on_wait else []
                    if len(waits) > mw:
                        n_extra = len(waits) - mw
                        for i in range(0, n_extra, mw):
                            nop = mybir.InstNoOp(
                                name=self.nc.get_next_instruction_name(),
                                ins=[],
                                outs=[],
                            )
                            nop.engine = inst.engine
                            nop.sync_info = mybir.SyncInfo(
                                on_wait=waits[i : min(i + mw, n_extra)],
                                on_update=[],
                            )
                            self.nc.register_instruction(nop, overwrite=True)
                            new_insts.append(nop)
                        inst.sync_info = mybir.SyncInfo(
                            on_wait=waits[n_extra:],
                            on_update=list(si.on_update or []),
                        )
                    new_insts.append(inst)
                blk.instructions = new_insts
            return ret

        # The tile-exit drain gets the same treatment but must stay last in
        # its engine stream, so split it during emission instead.
        def _drain_and_barrier(self, tick_clock, wait_clock):
            drain_inst = self.nc.sync.drain()
            wait_clock.add_sem_waits(
                drain_inst.ins, ScopedClock({None: tick_clock.global_clock})
            )
            si = drain_inst.ins.sync_info
            waits = list(si.on_wait) if si and si.on_wait else []
            if len(waits) > MAX_DRAIN_WAITS:
                drain_inst.ins.sync_info = mybir.SyncInfo(
                    on_wait=waits[:MAX_DRAIN_WAITS],
                    on_update=list(si.on_update or []),
                )
                for i in range(MAX_DRAIN_WAITS, len(waits), MAX_DRAIN_WAITS):
                    nop = self.nc.sync.nop()
                    nop.ins.sync_info = mybir.SyncInfo(
                        on_wait=waits[i : i + MAX_DRAIN_WAITS], on_update=[]
                    )
            self.nc.all_engine_barrier()
            assert self.sems is not None
            popped = self.nc._tile_sem_poison_stack.pop()
            assert popped is self._sem_poison
            self.nc.clear_and_free_semaphores(
                list(self.sems.allocated().values())
            )
            self.nc.all_engine_barrier()

    f32 = mybir.dt.float32
    bf16 = mybir.dt.bfloat16
    EXP = mybir.ActivationFunctionType.Exp
    MUL = mybir.AluOpType.mult
    ADD = mybir.AluOpType.add

    nc = bass.Bass("TRN2", num_devices=N_CORES)

    x_ext = nc.declare_dram_parameter("x", [L, E], bf16, isOutput=False)
    wq_ext = nc.declare_dram_parameter("wq", [E, HPC * HD], bf16, isOutput=False)
    wk_ext = nc.declare_dram_parameter("wk", [E, HD], bf16, isOutput=False)
    wv_ext = nc.declare_dram_parameter("wv", [E, HD], bf16, isOutput=False)
    rct_ext = nc.declare_dram_parameter("rct", [HD, L], bf16, isOutput=False)
    rst_ext = nc.declare_dram_parameter("rst", [HD, L], bf16, isOutput=False)
    permt_ext = nc.declare_dram_parameter("permt", [HD, HD], bf16, isOutput=False)
    mask_ext = nc.declare_dram_parameter("mask", [128, 384], bf16, isOutput=False)
    out_ext = nc.declare_dram_parameter("out", [HPC * L, HD + 1], f32, isOutput=True)
    import os
    DEBUG = bool(os.environ.get("KERNEL_DEBUG"))
    if DEBUG:
        dbg_mask_ext = nc.declare_dram_parameter(
            "dbg_mask", [128, 512], bf16, isOutput=True
        )
        dbg_pt_ext = nc.declare_dram_parameter(
            "dbg_pt", [128, 1024], bf16, isOutput=True
        )
        dbg_pt1_ext = nc.declare_dram_parameter(
            "dbg_pt1", [128, 1024], bf16, isOutput=True
        )
        dbg_ob_ext = nc.declare_dram_parameter(
            "dbg_ob", [2, 128, 258], f32, isOutput=True
        )

    with PatchedTileContext(nc) as tc:
        with (
            tc.tile_pool(name="const", bufs=1) as constp,
            tc.tile_pool(name="un", bufs=2) as unp,
            tc.tile_pool(name="rot", bufs=2) as rotp,
            tc.tile_pool(name="qt", bufs=2) as qtp,
            tc.tile_pool(name="pt", bufs=6) as ptp,
            tc.tile_pool(name="ost", bufs=3) as ostp,
            tc.tile_pool(name="pbig", bufs=3, space="PSUM") as pbig,
            tc.tile_pool(name="poutA", bufs=1, space="PSUM") as poutpA,
            tc.tile_pool(name="poutB", bufs=1, space="PSUM") as poutpB,
        ):
            # ---- constants: wk/wv first so xT transposes start early ----
            wk_sb = constp.tile([128, EC, HD], bf16, tag="wk")
            nc.sync.dma_start(
                out=wk_sb[:], in_=wk_ext.rearrange("(c p) d -> p c d", p=128)
            )
            wv_sb = constp.tile([128, EC, HD], bf16, tag="wv")
            nc.sync.dma_start(
                out=wv_sb[:], in_=wv_ext.rearrange("(c p) d -> p c d", p=128)
            )

            xt = constp.tile([128, EC, L], bf16, tag="xt")
            vones = constp.tile([128, NJ, HD + 1], bf16, tag="vones")
            kt = constp.tile([128, L], bf16, tag="kt")

            # ---- xT via hardware DMA transpose (per quarter, per e-chunk),
            # alternating between the two HWDGE queues (SP and ACT) ----
            def emit_transposes(h2):
                for ec in range(EC):
                    nc.sync.dma_start(
                        out=xt[:, ec, 1024 * h2 : 1024 * (h2 + 1)],
                        in_=x_ext[
                            1024 * h2 : 1024 * (h2 + 1),
                            128 * ec : 128 * (ec + 1),
                        ],
                        transpose=True,
                    )

            emit_transposes(0)
            wq_sb = constp.tile([128, EC, HPC * HD], bf16, tag="wq")
            nc.sync.dma_start(
                out=wq_sb[:], in_=wq_ext.rearrange("(c p) d -> p c d", p=128)
            )
            emit_transposes(1)
            rct_sb = constp.tile([128, L], bf16, tag="rct")
            nc.sync.dma_start(out=rct_sb[:], in_=rct_ext[:])
            rst_sb = constp.tile([128, L], bf16, tag="rst")
            nc.sync.dma_start(out=rst_sb[:], in_=rst_ext[:])
            permt_sb = constp.tile([128, 128], bf16, tag="permt")
            nc.sync.dma_start(out=permt_sb[:], in_=permt_ext[:])
            mask_sb = constp.tile([128, 384], bf16, tag="mask")
            nc.sync.dma_start(out=mask_sb[:], in_=mask_ext[:])
            nc.gpsimd.memset(vones[:, :, HD : HD + 1], 1.0)

            # ACT is the bottleneck engine (exp); keep all evacs on DVE/Pool.
            def evac_dve(dst_ap, src_ap):
                nc.vector.tensor_copy(dst_ap, src_ap)

            def k_half(h2):
                # K^T: d-major, 2 accum groups of 512 cols in one psum tile
                pk = pbig.tile([128, 1024], f32, tag="big", name=f"pk{h2}")
                for q in range(2):
                    w = 1024 * h2 + 512 * q
                    for ec in range(EC):
                        nc.tensor.matmul(
                            pk[:, 512 * q : 512 * (q + 1)],
                            wk_sb[:, ec, :],
                            xt[:, ec, w : w + 512],
                            start=(ec == 0),
                            stop=(ec == EC - 1),
                        )
                evac_dve(kt_un[:, 1024 * h2 : 1024 * (h2 + 1)], pk[:])

            def v_half(h2):
                # V rows: stationary = xt chunk, 8 j-block groups per tile
                pv = pbig.tile([128, 1024], f32, tag="big", name=f"pv{h2}")
                for m in range(8):
                    lb = 8 * h2 + m
                    for ec in range(EC):
                        nc.tensor.matmul(
                            pv[:, 128 * m : 128 * (m + 1)],
                            xt[:, ec, 128 * lb : 128 * (lb + 1)],
                            wv_sb[:, ec, :],
                            start=(ec == 0),
                            stop=(ec == EC - 1),
                            skip_group_check=True,
                        )
                    if m % 4 == 3:
                        evac_dve(
                            vones[:, lb - 3 : lb + 1, 0:HD],
                            pv[:, 128 * (m - 3) : 128 * (m + 1)],
                        )

            def rope_units(src_un, dst, rot_sb):
                # dst = src*Rc + (PermT.T @ src)*Rs, bf16 d-major, as
                # independently emittable units (pipeline fill work).
                # Chunked per 1024 cols so the consumer's first i-groups
                # unblock after chunk 0's add.
                def mul1(ch):
                    def f():
                        sl = slice(1024 * ch, 1024 * (ch + 1))
                        # Pool: dst = src * Rc (independent of the perm)
                        nc.gpsimd.tensor_tensor(
                            dst[:, sl], src_un[:, sl], rct_sb[:, sl], op=MUL
                        )
                    return f

                def permch(ch):
                    def f():
                        rp = pbig.tile([128, 1024], f32, tag="big", name="rp")
                        for q in range(2):
                            sl = slice(
                                1024 * ch + 512 * q, 1024 * ch + 512 * (q + 1)
                            )
                            nc.tensor.matmul(
                                rp[:, 512 * q : 512 * (q + 1)],
                                permt_sb[:],
                                src_un[:, sl],
                                start=True,
                                stop=True,
                            )
                        # fused evac: rot = psum * Rs (DVE)
                        nc.vector.tensor_tensor(
                            rot_sb[:, 1024 * ch : 1024 * (ch + 1)],
                            rp[:],
                            rst_sb[:, 1024 * ch : 1024 * (ch + 1)],
                            op=MUL,
                        )
                    return f

                def add(ch):
                    def f():
                        sl = slice(1024 * ch, 1024 * (ch + 1))
                        nc.vector.tensor_tensor(
                            dst[:, sl], dst[:, sl], rot_sb[:, sl], op=ADD
                        )
                    return f

                return [mul1(0), permch(0), mul1(1), permch(1), add(0), add(1)]

            # ---- Q projection emission units (also used as pipeline fill) --
            def q_proj_units(hl, qun_tile):
                units = []
                for ch in range(2):
                    def mk(ch=ch):
                        pk = pbig.tile(
                            [128, 1024], f32, tag="big", name=f"pq{hl}_{ch}"
                        )
                        for q in range(2):
                            w = 1024 * ch + 512 * q
                            for ec in range(EC):
                                nc.tensor.matmul(
                                    pk[:, 512 * q : 512 * (q + 1)],
                                    wq_sb[:, ec, 128 * hl : 128 * (hl + 1)],
                                    xt[:, ec, w : w + 512],
                                    start=(ec == 0),
                                    stop=(ec == EC - 1),
                                )
                        evac_dve(
                            qun_tile[:, 1024 * ch : 1024 * (ch + 1)], pk[:]
                        )
                    units.append(mk)
                return units

            # ---- prefix: K/V/Q0 interleaved with the xT transposes ----
            kt_un = unp.tile([128, L], bf16, tag="un")
            qun = unp.tile([128, L], bf16, tag="un", name="qun0")
            q0_units = q_proj_units(0, qun)
            k_half(0)
            v_half(0)
            q0_units[0]()
            k_half(1)
            v_half(1)
            q0_units[1]()
            krot = rotp.tile([128, L], bf16, tag="rot", name="krot")
            for u in rope_units(kt_un, kt, krot):
                u()

            # ---- attention per head with lookahead-2 pipeline ----
            LOOKAHEAD = 3

            def attention(hl, qt_t, fill_units):
                # tp list: (g, t0, nblocks, is_first, is_last)
                tps = []
                for g in range(NG):
                    n_t = 2 * g + 2
                    for t0 in range(0, n_t, 4):
                        nb = min(4, n_t - t0)
                        tps.append((g, t0, nb, t0 == 0, t0 + nb == n_t))
                n = len(tps)
                sc_tiles = [None] * n
                pt_tiles = [None] * n
                pout_tiles = {}
                fill = list(fill_units)
                fill_start = max(0, n - 2 - len(fill))

                def emit_sc(i):
                    g, t0, nb, _, is_last = tps[i]
                    n_t = 2 * g + 2
                    sc = pbig.tile([128, 1024], f32, tag="big", name=f"sc{i}")
                    for s in range(nb):
                        t = t0 + s
                        if t == n_t - 1:
                            # odd diagonal block: half0 rows are fully
                            # masked; compute only the live 128 i-columns
                            nc.tensor.matmul(
                                sc[:, 256 * s : 256 * s + 128],
                                kt[:, 128 * t : 128 * (t + 1)],
                                qt_t[:, 256 * g + 128 : 256 * (g + 1)],
                                start=True,
                                stop=True,
                            )
                        else:
                            nc.tensor.matmul(
                                sc[:, 256 * s : 256 * (s + 1)],
                                kt[:, 128 * t : 128 * (t + 1)],
                                qt_t[:, 256 * g : 256 * (g + 1)],
                                start=True,
                                stop=True,
                            )
                    sc_tiles[i] = sc

                for i in range(-LOOKAHEAD, n):
                    j = i + LOOKAHEAD
                    if j < n:
                        emit_sc(j)
                    if i >= fill_start and fill:
                        fill.pop(0)()
                    if i < 0:
                        continue
                    g, t0, nb, is_first, is_last = tps[i]
                    w = 256 * nb - (128 if is_last else 0)
                    pt_t = ptp.tile([128, 1024], bf16, tag="pt")
                    nc.scalar.activation(
                        pt_t[:, 0:w], sc_tiles[i][:, 0:w], EXP, scale=SCALE
                    )
                    if is_last:
                        # mask the two diagonal j-blocks (last 384 used cols)
                        nc.vector.tensor_tensor(
                            pt_t[:, w - 384 : w],
                            pt_t[:, w - 384 : w],
                            mask_sb[:],
                            op=MUL,
                        )
                    if DEBUG and hl == 0 and i == 0:
                        nc.sync.dma_start(out=dbg_mask_ext[:], in_=mask_sb[:])
                        nc.sync.dma_start(
                            out=dbg_pt_ext[:, 0:w], in_=pt_t[:, 0:w]
                        )
                    if DEBUG and hl == 0 and i == 1:
                        nc.sync.dma_start(
                            out=dbg_pt1_ext[:, 0:w], in_=pt_t[:, 0:w]
                        )
                    pt_tiles[i] = pt_t
                    sc_tiles[i] = None
                    if is_first:
                        pout_tiles[g] = (
                            poutpA.tile(
                                [128, 512], f32, tag="poA", name=f"poA_{hl}_{g}"
                            ),
                            poutpB.tile(
                                [128, 512], f32, tag="poB", name=f"poB_{hl}_{g}"
                            ),
                        )
                    po = pout_tiles[g]
                    n_t = 2 * g + 2
                    for s in range(nb):
                        t = t0 + s
                        if t == n_t - 1:
                            # odd diagonal block: packed live half1 only
                            nc.tensor.matmul(
                                po[1][:, 0 : HD + 1],
                                pt_t[:, 256 * s : 256 * s + 128],
                                vones[:, t, :],
                                start=(t == 0),
                                stop=True,
                                skip_group_check=True,
                            )
                            continue
                        for half in range(2):
                            nc.tensor.matmul(
                                po[half][:, 0 : HD + 1],
                                pt_t[:, 256 * s + 128 * half : 256 * s + 128 * (half + 1)],
                                vones[:, t, :],
                                start=(t == 0),
                                stop=(t == n_t - 1 - (1 - half) and t != n_t - 1),
                                skip_group_check=True,
                            )
                    pt_tiles[i] = None
                    if is_last:
                        ob = ostp.tile([128, 2 * (HD + 1)], f32, tag="ob")
                        evac_dve(ob[:, 0 : HD + 1], po[0][:, 0 : HD + 1])
                        evac_dve(ob[:, HD + 1 : 2 * (HD + 1)], po[1][:, 0 : HD + 1])
                        if DEBUG and hl == 0 and g < 2:
                            nc.sync.dma_start(
                                out=dbg_ob_ext[g, :, :], in_=ob[:]
                            )
                        for half in range(2):
                            row0 = L * hl + 256 * g + 128 * half
                            nc.sync.dma_start(
                                out=out_ext[row0 : row0 + 128, :],
                                in_=ob[:, 129 * half : 129 * (half + 1)],
                            )
                        del pout_tiles[g]
                # leftover fill units (next head's remaining proj work)
                for u in fill:
                    u()

            # head 0 rope (prefix)
            qt_cur = qtp.tile([128, L], bf16, tag="qt", name="qt0")
            qrot = rotp.tile([128, L], bf16, tag="rot", name="qrot0")
            for u in rope_units(qun, qt_cur, qrot):
                u()

            for hl in range(HPC):
                if hl + 1 < HPC:
                    qun_next = unp.tile(
                        [128, L], bf16, tag="un", name=f"qun{hl + 1}"
                    )
                    qt_next = qtp.tile(
                        [128, L], bf16, tag="qt", name=f"qt{hl + 1}"
                    )
                    qrot_next = rotp.tile(
                        [128, L], bf16, tag="rot", name=f"qrot{hl + 1}"
                    )
                    fill_units = q_proj_units(hl + 1, qun_next) + rope_units(
                        qun_next, qt_next, qrot_next
                    )
                else:
                    qt_next = None
                    fill_units = []
                attention(hl, qt_cur, fill_units)
                qt_cur = qt_next
    return nc


def _get_program():
    if "nc" not in _CACHE:
        _ensure_ntff_hook()
        _CACHE["nc"] = _build_program()
    return _CACHE["nc"]


def kernel(x, Wq, Wk, Wv, _trace=False):
    _ensure_ntff_hook()
    from concourse.bass_utils import run_bass_kernel_spmd

    nc = _get_program()
    rct, rst, permt, mask = _host_tables()
    xb = [
        np.ascontiguousarray(np.asarray(x[b]).astype(BF16)) for b in range(B)
    ]
    wq_bf = np.asarray(Wq).astype(BF16)
    wk_bf = np.ascontiguousarray(np.asarray(Wk).astype(BF16))
    wv_bf = np.ascontiguousarray(np.asarray(Wv).astype(BF16))
    in_maps = []
    for c in range(N_CORES):
        b, hq = divmod(c, HPC)
        in_maps.append(
            {
                "x": xb[b],
                "wq": np.ascontiguousarray(
                    wq_bf[:, HPC * HD * hq : HPC * HD * (hq + 1)]
                ),
                "wk": wk_bf,
                "wv": wv_bf,
                "rct": rct,
                "rst": rst,
                "permt": permt,
                "mask": mask,
            }
        )
    res = run_bass_kernel_spmd(
        nc, in_maps, list(range(N_CORES)), trace=_trace
    )
    out = np.empty((B, L, NH * HD), np.float32)
    for c in range(N_CORES):
        b, hq = divmod(c, HPC)
        raw = res.results[c]["out"].reshape(HPC, L, HD + 1)
        vals = raw[:, :, :HD] / raw[:, :, HD : HD + 1]  # (4, L, 128)
        out[b, :, HPC * HD * hq : HPC * HD * (hq + 1)] = (
            vals.transpose(1, 0, 2).reshape(L, HPC * HD)
        )
    if _trace:
        return out, res
    return out
